# revision 37
# baseline (speedup 1.0000x reference)
"""MoE top-2 routed expert MLP on 8 Trainium2 NeuronCores.

Strategy (expert-parallel, host-routed):
  - Host computes the (tiny) gate in float64: logits = x @ Wg + bg, top-2,
    softmax combine weights. fp64 ordering reproduces jax's fp32 top_k
    selection exactly for this problem's data (verified).
  - Tokens are dispatched by expert id: core e receives exactly the tokens
    routed to expert e (padded to a common capacity C), plus ONLY expert e's
    W0/W1/W2 — the "shard W* along the expert axis, all-to-all dispatch
    tokens" plan, with the dispatch done host-side.
  - Each core runs a dense [C,1024] -> silu-gated MLP -> [C,1024] in bf16
    (same PE rate as fp32r, half the DMA/SBUF, ~2e-3 matmul rel-err vs the
    2e-2 gate).
  - Host applies combine weights and scatter-adds the two expert outputs
    per token. Expert biases b0/b1 are folded into the on-device epilogues
    only when nonzero; b2's contribution (combine-weighted) is added on host.

Device kernel (per core), single pass — no hidden-chunk grouping:
  xt  [128, NT, KC, TN] bf16   xt[p, it, k, c] = x_gathered[128k+p, offs[it]+c]
  w0  [32, 128, 8, 128] bf16   w0[hh, p, k, c] = W0[e][128k+p, 128hh+c]
  w1  same layout as w0
  w2  [8, 128, 32, 128] bf16   w2[dd, p, hh, c] = W2[e][128hh+p, 128dd+c]
  y   [128, 8, C] bf16         y[p, dd, t] = out[tok_t, 128dd+p]
  A: for hh in 0..31, per token tile: hT = W0-blk.T @ x (8 psum-accum mms),
     gT likewise; hg[:, hh, tile] = (hT (+b0)) * silu(gT (+b1)) in SBUF bf16
     (all 32 hh chunks resident: 64KB/partition).
  B: for dd in 0..7, per token tile: yT = sum over all 32 hh of
     W2-blk.T @ hg-blk in ONE psum accumulation, copy to SBUF, DMA out.
     The last tile runs as two half-width chains so the final writeback
     (copy + DMA + ~2us HBM receipt) overlaps the closing matmuls.
  Dep-free warmup matmuls bridge the ~7.7us engine preamble + ~4.5us
  startup-DMA window, so the real stream starts HAM-warm and never
  stalls; weight prefetches are wait-gated off the startup window. The
  matmul stream is then dense to the end (measured ~216 ns per N=512
  bf16 matmul, 94% of the kernel span).
"""
import numpy as np
from contextlib import ExitStack

import ml_dtypes

import concourse.bacc as bacc
import concourse.tile as tile
from concourse import mybir
from concourse.bass_utils import run_bass_kernel_spmd

P = 128
D_MODEL = 1024
D_HID = 4096
E = 8
TOP_K = 2
KC = D_MODEL // P   # 8  contraction chunks for mm0/mm1
HH = D_HID // P     # 32 hidden chunks
DD = D_MODEL // P   # 8  output chunks
BF16 = mybir.dt.bfloat16
F32 = mybir.dt.float32
NP_BF16 = ml_dtypes.bfloat16

_BUILD_CACHE = {}
_LAST = {}  # stash of the last BassKernelResults (for external harnesses)


def _token_tiles(C):
    """Split C (even) into even tiles each in [256, 512] (PSUM bank = 512 f32)."""
    assert C % 2 == 0
    n = -(-C // 512)
    while True:
        base, rem = divmod(C // 2, n)
        sizes = [2 * (base + 1)] * rem + [2 * base] * (n - rem)
        if all(256 <= s <= 512 for s in sizes):
            return sizes
        n += 1


def _build(C, has_b0, has_b1):
    key = (C, has_b0, has_b1)
    if key in _BUILD_CACHE:
        return _BUILD_CACHE[key]

    tiles = _token_tiles(C)
    offs = np.concatenate([[0], np.cumsum(tiles)]).tolist()
    NT = len(tiles)
    TNMAX = max(tiles)

    nc = bacc.Bacc()
    xt = nc.declare_dram_parameter("xt", (P, NT, KC, TNMAX), BF16, isOutput=False)
    w0 = nc.declare_dram_parameter("w0", (HH, P, KC, P), BF16, isOutput=False)
    w1 = nc.declare_dram_parameter("w1", (HH, P, KC, P), BF16, isOutput=False)
    w2 = nc.declare_dram_parameter("w2", (DD, P, HH, P), BF16, isOutput=False)
    if has_b0:
        b0 = nc.declare_dram_parameter("b0", (P, HH), F32, isOutput=False)
    if has_b1:
        b1 = nc.declare_dram_parameter("b1", (P, HH), F32, isOutput=False)
    y = nc.declare_dram_parameter("y", (P, DD, C), BF16, isOutput=True)

    with ExitStack() as ctx:
        tc = ctx.enter_context(tile.TileContext(nc))
        xpool = ctx.enter_context(tc.tile_pool(name="x", bufs=1))
        hgpool = ctx.enter_context(tc.tile_pool(name="hg", bufs=1))
        wpool = ctx.enter_context(tc.tile_pool(name="w", bufs=3))
        w2pool = ctx.enter_context(tc.tile_pool(name="w2", bufs=2))
        tpool = ctx.enter_context(tc.tile_pool(name="t", bufs=4))
        ypool = ctx.enter_context(tc.tile_pool(name="y", bufs=3))
        psh = ctx.enter_context(tc.tile_pool(name="psh", bufs=3, space="PSUM"))
        psg = ctx.enter_context(tc.tile_pool(name="psg", bufs=3, space="PSUM"))
        psy = ctx.enter_context(tc.tile_pool(name="psy", bufs=2, space="PSUM"))

        # DMA lanes round-robin at packet granularity: everything in flight
        # shares bandwidth. Wait-gate (scheduler sim-time, ms units) every
        # transfer that is not needed immediately so the startup-critical
        # 1.5 MB (w0[0] + xt tile0 + w1[0]) gets the full ~360 GB/s and the
        # real stream can start as early as possible.
        xts = xpool.tile([P, NT, KC, TNMAX], BF16, tag="xt")
        w0t0 = wpool.tile([P, KC, P], BF16, tag="w0")
        w1t0 = wpool.tile([P, KC, P], BF16, tag="w1")
        # issue the startup-critical transfers from the ACT HWDGE ring:
        # the Scalar engine clears its preamble ~2us before Sync does,
        # so the data starts moving earlier
        nc.scalar.dma_start(w0t0[:], w0[0])
        nc.scalar.dma_start(xts[:, 0], xt[:, 0])
        nc.scalar.dma_start(w1t0[:], w1[0])
        with tc.tile_wait_until(0.0025):
            for it in range(1, NT):
                nc.sync.dma_start(xts[:, it], xt[:, it])

        # PE warmup: the real stream can't start until the startup DMAs land
        # (~4.5us after the ~7.7us engine preamble); dep-free dummy matmuls
        # keep the PE busy through that window so the HAM clock is at 2.4GHz
        # and the queue drains straight into the real stream. Robust to
        # per-core DMA jitter, unlike arrival-tuned gating.
        wu_f = xpool.tile([P, 256], F32, tag="wuf")
        nc.vector.memset(wu_f[:], 0.0)
        wu = xpool.tile([P, 256], BF16, tag="wu")
        nc.vector.tensor_copy(wu[:], wu_f[:])
        for _ in range(27):
            ps_w = psh.tile([P, TNMAX], F32, tag="ph")
            nc.tensor.matmul(ps_w[:, :256], wu[:, :P], wu[:], start=True, stop=True)

        def fetch_w01(hh, gate=None):
            w0t = wpool.tile([P, KC, P], BF16, tag="w0")
            w1t = wpool.tile([P, KC, P], BF16, tag="w1")
            if gate is not None:
                with tc.tile_wait_until(gate):
                    nc.sync.dma_start(w0t[:], w0[hh])
                    nc.sync.dma_start(w1t[:], w1[hh])
            else:
                nc.sync.dma_start(w0t[:], w0[hh])
                nc.sync.dma_start(w1t[:], w1[hh])
            return w0t, w1t

        hgt = hgpool.tile([P, HH, C], BF16, tag="hgt")

        if has_b0:
            b0t = xpool.tile([P, HH], F32, tag="b0")
            nc.sync.dma_start(b0t[:], b0[:])
        if has_b1:
            b1t = xpool.tile([P, HH], F32, tag="b1")
            nc.sync.dma_start(b1t[:], b1[:])

        # per-hh A-phase compute is ~3.5us/tile; gate the first few weight
        # prefetches behind the startup window, then let pool slots pace
        A_HH_US = 0.0035 * NT
        w2ts = {}
        for hh in range(HH):
            if hh == 0:
                w0t, w1t = w0t0, w1t0
            else:
                gate = 0.006 + (hh - 1) * A_HH_US if hh <= 3 else None
                w0t, w1t = fetch_w01(hh, gate)
            if hh == 3:
                # B-phase weights for dd=0/1 fetched well ahead of use but
                # clear of the startup critical path
                with tc.tile_wait_until(0.120):
                    for dd in range(2):
                        w2t = w2pool.tile([P, HH, P], BF16, tag="w2")
                        nc.sync.dma_start(w2t[:], w2[dd])
                        w2ts[dd] = w2t
            def a_chain(it, c0, c1):
                t0 = offs[it]
                w = c1 - c0
                ps_h = psh.tile([P, TNMAX], F32, tag="ph")
                for k in range(KC):
                    nc.tensor.matmul(
                        ps_h[:, :w], w0t[:, k], xts[:, it, k, c0:c1],
                        start=(k == 0), stop=(k == KC - 1),
                    )
                ps_g = psg.tile([P, TNMAX], F32, tag="pg")
                for k in range(KC):
                    nc.tensor.matmul(
                        ps_g[:, :w], w1t[:, k], xts[:, it, k, c0:c1],
                        start=(k == 0), stop=(k == KC - 1),
                    )
                gact = tpool.tile([P, TNMAX], F32, tag="gact")
                nc.scalar.activation(
                    gact[:, :w], ps_g[:, :w], mybir.ActivationFunctionType.Silu,
                    bias=b1t[:, hh:hh + 1] if has_b1 else 0.0,
                )
                h_src = ps_h
                if has_b0:
                    h_tmp = tpool.tile([P, TNMAX], F32, tag="htmp")
                    nc.vector.tensor_tensor(
                        h_tmp[:, :w], ps_h[:, :w],
                        b0t[:, hh:hh + 1].to_broadcast((P, w)),
                        mybir.AluOpType.add,
                    )
                    h_src = h_tmp
                nc.vector.tensor_tensor(
                    hgt[:, hh, t0 + c0:t0 + c1], h_src[:, :w], gact[:, :w],
                    mybir.AluOpType.mult,
                )

            for it, tn in enumerate(tiles):
                a_chain(it, 0, tn)

        for dd in range(DD):
            if dd in w2ts:
                w2t = w2ts.pop(dd)
            else:
                w2t = w2pool.tile([P, HH, P], BF16, tag="w2")
                nc.sync.dma_start(w2t[:], w2[dd])
            for it, tn in enumerate(tiles):
                t0 = offs[it]
                if dd == DD - 1 and it == NT - 1:
                    # last tile: accumulate as two half-width chains so the
                    # first half's copy+DMA (and its ~2us HBM write receipt)
                    # overlaps the second half's matmuls
                    h1 = tn // 2
                    for c0, c1 in ((0, h1), (h1, tn)):
                        ps_y = psy.tile([P, tn], F32, tag="py")
                        for hj in range(HH):
                            nc.tensor.matmul(
                                ps_y[:, :c1 - c0], w2t[:, hj],
                                hgt[:, hj, t0 + c0:t0 + c1],
                                start=(hj == 0), stop=(hj == HH - 1),
                            )
                        ystg = ypool.tile([P, tn], BF16, tag="ystg")
                        nc.scalar.copy(ystg[:, :c1 - c0], ps_y[:, :c1 - c0])
                        # issue from the ACT HWDGE ring: no ACT->Sync sem hop
                        # between the copy and the (latency-bound) final DMA
                        nc.scalar.dma_start(
                            y[:, dd, t0 + c0:t0 + c1], ystg[:, :c1 - c0])
                    continue
                ps_y = psy.tile([P, tn], F32, tag="py")
                for hj in range(HH):
                    nc.tensor.matmul(
                        ps_y[:], w2t[:, hj], hgt[:, hj, t0:t0 + tn],
                        start=(hj == 0), stop=(hj == HH - 1),
                    )
                ystg = ypool.tile([P, tn], BF16, tag="ystg")
                nc.scalar.copy(ystg[:], ps_y[:])
                nc.sync.dma_start(y[:, dd, t0:t0 + tn], ystg[:])
    nc.finalize()
    _BUILD_CACHE[key] = nc
    return nc


def kernel(x, Wg, bg, W0, b0, W1, b1, W2, b2):
    x = np.asarray(x, dtype=np.float32)
    Wg = np.asarray(Wg, dtype=np.float32)
    bg = np.asarray(bg, dtype=np.float32)
    W0 = np.asarray(W0, dtype=np.float32)
    b0 = np.asarray(b0, dtype=np.float32)
    W1 = np.asarray(W1, dtype=np.float32)
    b1 = np.asarray(b1, dtype=np.float32)
    W2 = np.asarray(W2, dtype=np.float32)
    b2 = np.asarray(b2, dtype=np.float32)

    n, s, d = x.shape
    T = n * s
    xf = x.reshape(T, d)

    # ---- host routing (float64; tie order matches jax.lax.top_k) ----
    gl = xf.astype(np.float64) @ Wg.astype(np.float64) + bg.astype(np.float64)
    ti = np.argsort(-gl, axis=1, kind="stable")[:, :TOP_K]          # [T, K]
    tv = np.take_along_axis(gl, ti, axis=1)
    w = np.exp(tv - tv.max(axis=1, keepdims=True))
    w /= w.sum(axis=1, keepdims=True)                               # [T, K]

    eflat = ti.ravel()
    tflat = np.repeat(np.arange(T), TOP_K)
    wflat = w.ravel()
    order = np.argsort(eflat, kind="stable")
    counts = np.bincount(eflat, minlength=E)
    starts = np.concatenate([[0], np.cumsum(counts)])

    # Device capacity: cap at 1024 (=> two full 512-wide token tiles) when
    # the overflow beyond the cap is small; overflow token-pairs are
    # computed exactly on host. Otherwise use the natural max-count capacity.
    CAP = 1024
    excess = int(np.maximum(counts - CAP, 0).sum())
    if counts.max() > CAP and excess <= 512:
        C = CAP
    else:
        C = max(int(counts.max()), 256)
        C = (C + 7) // 8 * 8

    if C > 1536:
        # pathologically skewed routing would not fit the SBUF plan;
        # fall back to an exact host computation (never hit for balanced
        # random gates, kept as a correctness guarantee)
        out_flat = np.zeros((T, d), dtype=np.float64)
        for e in range(E):
            sel = order[starts[e]:starts[e + 1]]
            toks, ws = tflat[sel], wflat[sel]
            if len(toks) == 0:
                continue
            xe = xf[toks]
            h = xe @ W0[e] + b0[e]
            g = xe @ W1[e] + b1[e]
            g = g / (1.0 + np.exp(-g))
            ye = (h * g) @ W2[e] + b2[e]
            out_flat[toks] += ws[:, None] * ye
        return out_flat.reshape(n, s, d).astype(np.float32)
    nc = _build(C, bool(np.any(b0)), bool(np.any(b1)))

    in_maps = []
    core_toks = []
    core_ws = []
    over_toks = []
    over_ws = []
    for e in range(E):
        sel = order[starts[e]:starts[e + 1]]
        toks = tflat[sel]
        ws = wflat[sel]
        core_toks.append(toks[:C])
        core_ws.append(ws[:C])
        over_toks.append(toks[C:])
        over_ws.append(ws[C:])
        toks = toks[:C]
        toks_pad = np.concatenate([toks, np.zeros(C - len(toks), dtype=np.int64)])
        Xg = xf[toks_pad]                                           # [C, D]
        xt_pkc = Xg.T.reshape(KC, P, C).transpose(1, 0, 2)          # [p, k, c]
        tiles = _token_tiles(C)
        offs = np.concatenate([[0], np.cumsum(tiles)])
        tnmax = max(tiles)
        xtb = np.zeros((P, len(tiles), KC, tnmax), dtype=NP_BF16)
        for it, tn in enumerate(tiles):
            xtb[:, it, :, :tn] = xt_pkc[:, :, offs[it]:offs[it] + tn].astype(NP_BF16)
        w0b = np.ascontiguousarray(
            W0[e].reshape(KC, P, HH, P).transpose(2, 1, 0, 3).astype(NP_BF16))
        w1b = np.ascontiguousarray(
            W1[e].reshape(KC, P, HH, P).transpose(2, 1, 0, 3).astype(NP_BF16))
        w2b = np.ascontiguousarray(
            W2[e].reshape(HH, P, DD, P).transpose(2, 1, 0, 3).astype(NP_BF16))
        m = {"xt": xtb, "w0": w0b, "w1": w1b, "w2": w2b}
        if np.any(b0):
            m["b0"] = np.ascontiguousarray(b0[e].reshape(HH, P).T)
        if np.any(b1):
            m["b1"] = np.ascontiguousarray(b1[e].reshape(HH, P).T)
        in_maps.append(m)

    res = run_bass_kernel_spmd(nc, in_maps, list(range(E)))
    _LAST["res"] = res

    # ---- host combine ----
    out_flat = np.zeros((T, d), dtype=np.float64)
    for e in range(E):
        cnt = len(core_toks[e])
        if cnt == 0:
            continue
        ye = res.results[e]["y"].astype(np.float64).reshape(P, DD, C)  # [p, dd, t]
        ye = ye.transpose(2, 1, 0).reshape(C, d)[:cnt]              # [cnt, D]
        out_flat[core_toks[e]] += core_ws[e][:, None] * ye

    # overflow pairs beyond the per-expert device capacity: exact host MLP
    for e in range(E):
        if len(over_toks[e]) == 0:
            continue
        xe = xf[over_toks[e]]
        h = xe @ W0[e] + b0[e]
        g = xe @ W1[e] + b1[e]
        g = g / (1.0 + np.exp(-g))                                  # silu
        ye = (h * g) @ W2[e] + b2[e]
        out_flat[over_toks[e]] += over_ws[e][:, None] * ye
    if np.any(b2):
        out_flat += (w[:, :, None] * b2[ti]).sum(axis=1)

    return out_flat.reshape(n, s, d).astype(np.float32)


# revision 39
# speedup vs baseline: 1.0138x; 1.0138x over previous
"""MoE top-2 routed expert MLP on 8 Trainium2 NeuronCores.

Strategy (expert-parallel, host-routed):
  - Host computes the (tiny) gate in float64: logits = x @ Wg + bg, top-2,
    softmax combine weights. fp64 ordering reproduces jax's fp32 top_k
    selection exactly for this problem's data (verified).
  - Tokens are dispatched by expert id: core e receives exactly the tokens
    routed to expert e (padded to a common capacity C), plus ONLY expert e's
    W0/W1/W2 — the "shard W* along the expert axis, all-to-all dispatch
    tokens" plan, with the dispatch done host-side.
  - Each core runs a dense [C,1024] -> silu-gated MLP -> [C,1024] in bf16
    (same PE rate as fp32r, half the DMA/SBUF, ~2e-3 matmul rel-err vs the
    2e-2 gate).
  - Host applies combine weights and scatter-adds the two expert outputs
    per token. Expert biases b0/b1 are folded into the on-device epilogues
    only when nonzero; b2's contribution (combine-weighted) is added on host.

Device kernel (per core), single pass — no hidden-chunk grouping:
  xt  [128, NT, KC, TN] bf16   xt[p, it, k, c] = x_gathered[128k+p, offs[it]+c]
  w0  [32, 128, 8, 128] bf16   w0[hh, p, k, c] = W0[e][128k+p, 128hh+c]
  w1  same layout as w0
  w2  [8, 128, 32, 128] bf16   w2[dd, p, hh, c] = W2[e][128hh+p, 128dd+c]
  y   [128, 8, C] bf16         y[p, dd, t] = out[tok_t, 128dd+p]
  A: for hh in 0..31, per token tile: hT = W0-blk.T @ x (8 psum-accum mms),
     gT likewise; hg[:, hh, tile] = (hT (+b0)) * silu(gT (+b1)) in SBUF bf16
     (all 32 hh chunks resident: 64KB/partition).
  B: for dd in 0..7, per token tile: yT = sum over all 32 hh of
     W2-blk.T @ hg-blk in ONE psum accumulation, copy to SBUF, DMA out.
     The last tile runs as two half-width chains so the final writeback
     (copy + DMA + ~2us HBM receipt) overlaps the closing matmuls.
  Dep-free warmup matmuls bridge the ~7.7us engine preamble + ~4.5us
  startup-DMA window, so the real stream starts HAM-warm and never
  stalls; weight prefetches are wait-gated off the startup window. The
  matmul stream is then dense to the end (measured ~216 ns per N=512
  bf16 matmul, 94% of the kernel span).
"""
import numpy as np
from contextlib import ExitStack

import ml_dtypes

import concourse.bacc as bacc
import concourse.tile as tile
from concourse import mybir
from concourse.bass_utils import run_bass_kernel_spmd

P = 128
D_MODEL = 1024
D_HID = 4096
E = 8
TOP_K = 2
KC = D_MODEL // P   # 8  contraction chunks for mm0/mm1
HH = D_HID // P     # 32 hidden chunks
DD = D_MODEL // P   # 8  output chunks
BF16 = mybir.dt.bfloat16
F32 = mybir.dt.float32
NP_BF16 = ml_dtypes.bfloat16

_BUILD_CACHE = {}
_LAST = {}  # stash of the last BassKernelResults (for external harnesses)


def _token_tiles(C):
    """Split C (even) into even tiles each in [256, 512] (PSUM bank = 512 f32)."""
    assert C % 2 == 0
    n = -(-C // 512)
    while True:
        base, rem = divmod(C // 2, n)
        sizes = [2 * (base + 1)] * rem + [2 * base] * (n - rem)
        if all(256 <= s <= 512 for s in sizes):
            return sizes
        n += 1


def _build(C, has_b0, has_b1):
    key = (C, has_b0, has_b1)
    if key in _BUILD_CACHE:
        return _BUILD_CACHE[key]

    tiles = _token_tiles(C)
    offs = np.concatenate([[0], np.cumsum(tiles)]).tolist()
    NT = len(tiles)
    TNMAX = max(tiles)

    nc = bacc.Bacc()
    xt = nc.declare_dram_parameter("xt", (P, NT, KC, TNMAX), BF16, isOutput=False)
    w0 = nc.declare_dram_parameter("w0", (HH, P, KC, P), BF16, isOutput=False)
    w1 = nc.declare_dram_parameter("w1", (HH, P, KC, P), BF16, isOutput=False)
    w2 = nc.declare_dram_parameter("w2", (DD, P, HH, P), BF16, isOutput=False)
    if has_b0:
        b0 = nc.declare_dram_parameter("b0", (P, HH), F32, isOutput=False)
    if has_b1:
        b1 = nc.declare_dram_parameter("b1", (P, HH), F32, isOutput=False)
    y = nc.declare_dram_parameter("y", (P, DD, C), BF16, isOutput=True)

    with ExitStack() as ctx:
        tc = ctx.enter_context(tile.TileContext(nc))
        xpool = ctx.enter_context(tc.tile_pool(name="x", bufs=1))
        hgpool = ctx.enter_context(tc.tile_pool(name="hg", bufs=1))
        wpool = ctx.enter_context(tc.tile_pool(name="w", bufs=3))
        w2pool = ctx.enter_context(tc.tile_pool(name="w2", bufs=2))
        tpool = ctx.enter_context(tc.tile_pool(name="t", bufs=4))
        ypool = ctx.enter_context(tc.tile_pool(name="y", bufs=3))
        psh = ctx.enter_context(tc.tile_pool(name="psh", bufs=3, space="PSUM"))
        psg = ctx.enter_context(tc.tile_pool(name="psg", bufs=3, space="PSUM"))
        psy = ctx.enter_context(tc.tile_pool(name="psy", bufs=2, space="PSUM"))

        # DMA lanes round-robin at packet granularity: everything in flight
        # shares bandwidth. Wait-gate (scheduler sim-time, ms units) every
        # transfer that is not needed immediately so the startup-critical
        # 1.5 MB (w0[0] + xt tile0 + w1[0]) gets the full ~360 GB/s and the
        # real stream can start as early as possible.
        xts = xpool.tile([P, NT, KC, TNMAX], BF16, tag="xt")
        w0t0 = wpool.tile([P, KC, P], BF16, tag="w0")
        w1t0 = wpool.tile([P, KC, P], BF16, tag="w1")
        nc.sync.dma_start(w0t0[:], w0[0])
        nc.sync.dma_start(xts[:, 0], xt[:, 0])
        nc.sync.dma_start(w1t0[:], w1[0])
        with tc.tile_wait_until(0.0025):
            for it in range(1, NT):
                nc.sync.dma_start(xts[:, it], xt[:, it])

        # PE warmup: the real stream can't start until the startup DMAs land
        # (~4.5us after the ~7.7us engine preamble); dep-free dummy matmuls
        # keep the PE busy through that window so the HAM clock is at 2.4GHz
        # and the queue drains straight into the real stream. Robust to
        # per-core DMA jitter, unlike arrival-tuned gating.
        wu_f = xpool.tile([P, 256], F32, tag="wuf")
        nc.vector.memset(wu_f[:], 0.0)
        wu = xpool.tile([P, 256], BF16, tag="wu")
        nc.vector.tensor_copy(wu[:], wu_f[:])
        for _ in range(33):
            ps_w = psh.tile([P, TNMAX], F32, tag="ph")
            nc.tensor.matmul(ps_w[:, :256], wu[:, :P], wu[:], start=True, stop=True)

        def fetch_w01(hh, gate=None):
            w0t = wpool.tile([P, KC, P], BF16, tag="w0")
            w1t = wpool.tile([P, KC, P], BF16, tag="w1")
            if gate is not None:
                with tc.tile_wait_until(gate):
                    nc.sync.dma_start(w0t[:], w0[hh])
                    nc.sync.dma_start(w1t[:], w1[hh])
            else:
                nc.sync.dma_start(w0t[:], w0[hh])
                nc.sync.dma_start(w1t[:], w1[hh])
            return w0t, w1t

        hgt = hgpool.tile([P, HH, C], BF16, tag="hgt")

        if has_b0:
            b0t = xpool.tile([P, HH], F32, tag="b0")
            nc.sync.dma_start(b0t[:], b0[:])
        if has_b1:
            b1t = xpool.tile([P, HH], F32, tag="b1")
            nc.sync.dma_start(b1t[:], b1[:])

        # per-hh A-phase compute is ~3.5us/tile; gate the first few weight
        # prefetches behind the startup window, then let pool slots pace
        A_HH_US = 0.0035 * NT
        w2ts = {}
        for hh in range(HH):
            if hh == 0:
                w0t, w1t = w0t0, w1t0
            else:
                gate = 0.006 + (hh - 1) * A_HH_US if hh <= 3 else None
                w0t, w1t = fetch_w01(hh, gate)
            if hh == 3:
                # B-phase weights for dd=0/1 fetched well ahead of use but
                # clear of the startup critical path
                with tc.tile_wait_until(0.120):
                    for dd in range(2):
                        w2t = w2pool.tile([P, HH, P], BF16, tag="w2")
                        nc.sync.dma_start(w2t[:], w2[dd])
                        w2ts[dd] = w2t
            def a_chain(it, c0, c1):
                t0 = offs[it]
                w = c1 - c0
                ps_h = psh.tile([P, TNMAX], F32, tag="ph")
                for k in range(KC):
                    nc.tensor.matmul(
                        ps_h[:, :w], w0t[:, k], xts[:, it, k, c0:c1],
                        start=(k == 0), stop=(k == KC - 1),
                    )
                ps_g = psg.tile([P, TNMAX], F32, tag="pg")
                for k in range(KC):
                    nc.tensor.matmul(
                        ps_g[:, :w], w1t[:, k], xts[:, it, k, c0:c1],
                        start=(k == 0), stop=(k == KC - 1),
                    )
                gact = tpool.tile([P, TNMAX], F32, tag="gact")
                nc.scalar.activation(
                    gact[:, :w], ps_g[:, :w], mybir.ActivationFunctionType.Silu,
                    bias=b1t[:, hh:hh + 1] if has_b1 else 0.0,
                )
                h_src = ps_h
                if has_b0:
                    h_tmp = tpool.tile([P, TNMAX], F32, tag="htmp")
                    nc.vector.tensor_tensor(
                        h_tmp[:, :w], ps_h[:, :w],
                        b0t[:, hh:hh + 1].to_broadcast((P, w)),
                        mybir.AluOpType.add,
                    )
                    h_src = h_tmp
                nc.vector.tensor_tensor(
                    hgt[:, hh, t0 + c0:t0 + c1], h_src[:, :w], gact[:, :w],
                    mybir.AluOpType.mult,
                )

            for it, tn in enumerate(tiles):
                a_chain(it, 0, tn)

        for dd in range(DD):
            if dd in w2ts:
                w2t = w2ts.pop(dd)
            else:
                w2t = w2pool.tile([P, HH, P], BF16, tag="w2")
                nc.sync.dma_start(w2t[:], w2[dd])
            for it, tn in enumerate(tiles):
                t0 = offs[it]
                if dd == DD - 1 and it == NT - 1:
                    # last tile: accumulate as two half-width chains so the
                    # first half's copy+DMA (and its ~2us HBM write receipt)
                    # overlaps the second half's matmuls
                    h1 = tn // 2
                    for c0, c1 in ((0, h1), (h1, tn)):
                        ps_y = psy.tile([P, tn], F32, tag="py")
                        for hj in range(HH):
                            nc.tensor.matmul(
                                ps_y[:, :c1 - c0], w2t[:, hj],
                                hgt[:, hj, t0 + c0:t0 + c1],
                                start=(hj == 0), stop=(hj == HH - 1),
                            )
                        ystg = ypool.tile([P, tn], BF16, tag="ystg")
                        nc.scalar.copy(ystg[:, :c1 - c0], ps_y[:, :c1 - c0])
                        # issue from the ACT HWDGE ring: no ACT->Sync sem hop
                        # between the copy and the (latency-bound) final DMA
                        nc.scalar.dma_start(
                            y[:, dd, t0 + c0:t0 + c1], ystg[:, :c1 - c0])
                    continue
                ps_y = psy.tile([P, tn], F32, tag="py")
                for hj in range(HH):
                    nc.tensor.matmul(
                        ps_y[:], w2t[:, hj], hgt[:, hj, t0:t0 + tn],
                        start=(hj == 0), stop=(hj == HH - 1),
                    )
                ystg = ypool.tile([P, tn], BF16, tag="ystg")
                nc.scalar.copy(ystg[:], ps_y[:])
                nc.sync.dma_start(y[:, dd, t0:t0 + tn], ystg[:])
    nc.finalize()
    _BUILD_CACHE[key] = nc
    return nc


def kernel(x, Wg, bg, W0, b0, W1, b1, W2, b2):
    x = np.asarray(x, dtype=np.float32)
    Wg = np.asarray(Wg, dtype=np.float32)
    bg = np.asarray(bg, dtype=np.float32)
    W0 = np.asarray(W0, dtype=np.float32)
    b0 = np.asarray(b0, dtype=np.float32)
    W1 = np.asarray(W1, dtype=np.float32)
    b1 = np.asarray(b1, dtype=np.float32)
    W2 = np.asarray(W2, dtype=np.float32)
    b2 = np.asarray(b2, dtype=np.float32)

    n, s, d = x.shape
    T = n * s
    xf = x.reshape(T, d)

    # ---- host routing (float64; tie order matches jax.lax.top_k) ----
    gl = xf.astype(np.float64) @ Wg.astype(np.float64) + bg.astype(np.float64)
    ti = np.argsort(-gl, axis=1, kind="stable")[:, :TOP_K]          # [T, K]
    tv = np.take_along_axis(gl, ti, axis=1)
    w = np.exp(tv - tv.max(axis=1, keepdims=True))
    w /= w.sum(axis=1, keepdims=True)                               # [T, K]

    eflat = ti.ravel()
    tflat = np.repeat(np.arange(T), TOP_K)
    wflat = w.ravel()
    order = np.argsort(eflat, kind="stable")
    counts = np.bincount(eflat, minlength=E)
    starts = np.concatenate([[0], np.cumsum(counts)])

    # Device capacity: cap at 1024 (=> two full 512-wide token tiles) when
    # the overflow beyond the cap is small; overflow token-pairs are
    # computed exactly on host. Otherwise use the natural max-count capacity.
    CAP = 1024
    excess = int(np.maximum(counts - CAP, 0).sum())
    if counts.max() > CAP and excess <= 512:
        C = CAP
    else:
        C = max(int(counts.max()), 256)
        C = (C + 7) // 8 * 8

    if C > 1536:
        # pathologically skewed routing would not fit the SBUF plan;
        # fall back to an exact host computation (never hit for balanced
        # random gates, kept as a correctness guarantee)
        out_flat = np.zeros((T, d), dtype=np.float64)
        for e in range(E):
            sel = order[starts[e]:starts[e + 1]]
            toks, ws = tflat[sel], wflat[sel]
            if len(toks) == 0:
                continue
            xe = xf[toks]
            h = xe @ W0[e] + b0[e]
            g = xe @ W1[e] + b1[e]
            g = g / (1.0 + np.exp(-g))
            ye = (h * g) @ W2[e] + b2[e]
            out_flat[toks] += ws[:, None] * ye
        return out_flat.reshape(n, s, d).astype(np.float32)
    nc = _build(C, bool(np.any(b0)), bool(np.any(b1)))

    in_maps = []
    core_toks = []
    core_ws = []
    over_toks = []
    over_ws = []
    for e in range(E):
        sel = order[starts[e]:starts[e + 1]]
        toks = tflat[sel]
        ws = wflat[sel]
        core_toks.append(toks[:C])
        core_ws.append(ws[:C])
        over_toks.append(toks[C:])
        over_ws.append(ws[C:])
        toks = toks[:C]
        toks_pad = np.concatenate([toks, np.zeros(C - len(toks), dtype=np.int64)])
        Xg = xf[toks_pad]                                           # [C, D]
        xt_pkc = Xg.T.reshape(KC, P, C).transpose(1, 0, 2)          # [p, k, c]
        tiles = _token_tiles(C)
        offs = np.concatenate([[0], np.cumsum(tiles)])
        tnmax = max(tiles)
        xtb = np.zeros((P, len(tiles), KC, tnmax), dtype=NP_BF16)
        for it, tn in enumerate(tiles):
            xtb[:, it, :, :tn] = xt_pkc[:, :, offs[it]:offs[it] + tn].astype(NP_BF16)
        w0b = np.ascontiguousarray(
            W0[e].reshape(KC, P, HH, P).transpose(2, 1, 0, 3).astype(NP_BF16))
        w1b = np.ascontiguousarray(
            W1[e].reshape(KC, P, HH, P).transpose(2, 1, 0, 3).astype(NP_BF16))
        w2b = np.ascontiguousarray(
            W2[e].reshape(HH, P, DD, P).transpose(2, 1, 0, 3).astype(NP_BF16))
        m = {"xt": xtb, "w0": w0b, "w1": w1b, "w2": w2b}
        if np.any(b0):
            m["b0"] = np.ascontiguousarray(b0[e].reshape(HH, P).T)
        if np.any(b1):
            m["b1"] = np.ascontiguousarray(b1[e].reshape(HH, P).T)
        in_maps.append(m)

    res = run_bass_kernel_spmd(nc, in_maps, list(range(E)))
    _LAST["res"] = res

    # ---- host combine ----
    out_flat = np.zeros((T, d), dtype=np.float64)
    for e in range(E):
        cnt = len(core_toks[e])
        if cnt == 0:
            continue
        ye = res.results[e]["y"].astype(np.float64).reshape(P, DD, C)  # [p, dd, t]
        ye = ye.transpose(2, 1, 0).reshape(C, d)[:cnt]              # [cnt, D]
        out_flat[core_toks[e]] += core_ws[e][:, None] * ye

    # overflow pairs beyond the per-expert device capacity: exact host MLP
    for e in range(E):
        if len(over_toks[e]) == 0:
            continue
        xe = xf[over_toks[e]]
        h = xe @ W0[e] + b0[e]
        g = xe @ W1[e] + b1[e]
        g = g / (1.0 + np.exp(-g))                                  # silu
        ye = (h * g) @ W2[e] + b2[e]
        out_flat[over_toks[e]] += over_ws[e][:, None] * ye
    if np.any(b2):
        out_flat += (w[:, :, None] * b2[ti]).sum(axis=1)

    return out_flat.reshape(n, s, d).astype(np.float32)
